# revision 2
# baseline (speedup 1.0000x reference)
"""Trainium2 Bass kernel for histogram_binning (windowed-cosine binning).

Reference computation (per element):
    d = x[k,i] - phis[i,j]
    out[k, i*L+j] = 0.5*cos(d)+0.5  if  -interval[i] < d <= interval[i]  else 0

v3 strategy (8 cores, data-parallel over batch):
  - Single fused custom DVE op does EVERYTHING in one 1x pass (6 ALU
    stages), writing fp16; no ACT sin pass, no separate d-pass:
        d  = Src1 - Src0          # s*x - s*phi   (stream pre-scaled by s)
        v  = d * d                # = B * (d_true/2)^2  since s = sqrt(B)/2
        c  = A - v                # linear minimax fit of cos(d/2)
        m  = v <= T               # T = (s*iv)^2  (per-partition scalar)
        out = (m*c)^2             # = m * cos^2(d/2) = m*(0.5cos d + 0.5)
  - Src1 (x) is delivered per batch-row via a [P, K, L] access pattern
    with inner stride 0 (STT struct), so one instruction covers K rows:
    FD = K*L = 4096 -> per-instruction overhead amortized to ~1.5%.
  - Output fp16 halves HBM write traffic (47us/core) below the DVE pass
    (~69us/core), so the kernel is DVE-bound near the 1-pass floor.
    Host upcasts to f32 (well within the 2e-2 rel-err gate: measured
    ~5e-4 from the linear fit + fp16 quantization).
  - Mask boundary: v <= T on the scaled stream flips vs the reference's
    fl(x-phi) <= iv only within ~2ulp of the window edge (a handful of
    elements in 67M) and at d == -iv exactly (measure zero).
"""

import math
import os

import numpy as np

import concourse.bacc as bacc
import concourse.mybir as mybir
from concourse import dve_ops
from concourse.bass_utils import run_bass_kernel_spmd
from concourse.dve_spec import (
    C0,
    C1,
    C2,
    One,
    Spec,
    Src0,
    Src1,
    Zero,
    _has_src1,
    lower,
)
from concourse.dve_uop import DveOpSpec
from concourse.tile import TileContext

B, M, L = 1024, 256, 256
N_CORES = 8
B_SHARD = B // N_CORES  # 128
HALF = 128  # features per partition-half
F32 = mybir.dt.float32
F16 = mybir.dt.float16
HALF_PI = float(np.pi / 2)

# minimax linear fit of cos(sqrt(t)) on t in [0, 0.25]:  cos(u) ~ A - B*u^2
FIT_A = 0.9996779888323271
FIT_B = 0.4896611490939912
SCALE_S = math.sqrt(FIT_B) / 2.0  # stream pre-scale: v = (s*d)^2 = B*(d/2)^2

_OPS_CACHE = {}


def _register_op(name, spec):
    """Register a custom DVE op under `name`, computing its uops sha."""
    if name in _OPS_CACHE:
        return _OPS_CACHE[name]
    for existing in dve_ops.OPS:
        if existing.name == name:
            _OPS_CACHE[name] = existing
            return existing
    if name not in dve_ops._SUB_OPCODE_FOR_NAME:
        row = max(dve_ops._SUB_OPCODE_FOR_NAME.values()) + 1
        assert row < 0x20, "no free custom-DVE opcode rows"
        dve_ops._SUB_OPCODE_FOR_NAME[name] = row
    shas = {}
    for ver in ("v3", "v4"):
        uops = lower(spec, ver=ver)
        shas[ver] = DveOpSpec(
            name=name,
            opcode=dve_ops.get_dve_sub_opcode(name),
            uops=uops,
            rd1_en=_has_src1(spec),
        ).sha(ver)
    op = dve_ops.DveOp(name, spec, subdim=False, uops_sha=shas)
    dve_ops.OPS.append(op)
    dve_ops.CUSTOM_DVE_SPECS[name] = spec
    _OPS_CACHE[name] = op
    return op


def _get_wincos_op():
    """out = (m*(A - v))^2 with v = (Src1-Src0)^2, m = v <= C0.
    Src0 = s*phi stream, Src1 = s*x (paged, inner stride 0),
    C0 (s0) = (s*iv)^2 per-partition, C1 (s1) = A literal."""
    d = Src1 - Src0
    v = d * d
    c = C1 - v
    m = v <= C0
    mc = m * c
    body = mc * mc

    def _ref(in0, in1, s0, s1, imm2):
        f = np.float32
        dd = (in1 - in0).astype(np.float32)
        vv = (dd * dd).astype(np.float32)
        mm = (vv <= s0).astype(np.float32)
        cc = (f(s1) - vv).astype(np.float32)
        r = (mm * cc).astype(np.float32)
        return (r * r).astype(np.float32)

    return _register_op("WINCOS_FUSED_ANT", Spec(body=body, reference=_ref))


def build_nc(variant="v3", K=16, num_devices=N_CORES, reps=1, **_unused):
    """Build the per-core Bass program (v3: single fused custom op)."""
    assert B_SHARD % K == 0
    n_chunks = B_SHARD // K

    nc = bacc.Bacc(
        "TRN2",
        target_bir_lowering=False,
        debug=False,
        enable_asserts=True,
        num_devices=num_devices,
    )
    # host-prescaled inputs: xs = s*x^T [M, B_SHARD], phs = s*phis [M, L],
    # thr = (s*interval)^2 [M]
    xs_d = nc.dram_tensor("xs", [M, B_SHARD], F32, kind="ExternalInput")
    ph_d = nc.dram_tensor("phs", [M, L], F32, kind="ExternalInput")
    th_d = nc.dram_tensor("thr", [M], F32, kind="ExternalInput")
    y_d = nc.dram_tensor("out", [B_SHARD, M * L], F16, kind="ExternalOutput")
    # out[k, (h*128+i)*256 + j] viewed as [h, i(part), k, j]
    yr = y_d.ap().rearrange("b (h i j) -> h i b j", h=2, i=HALF, j=L)
    thr = th_d.ap().rearrange("(h i one) -> h i one", h=2, one=1)
    xsr = xs_d.ap().rearrange("(h i) b -> h i b", h=2)
    phr = ph_d.ap().rearrange("(h i) j -> h i j", h=2)

    wincos = _get_wincos_op()

    with TileContext(nc) as tc:
        with (
            tc.tile_pool(name="const", bufs=1) as cpool,
            tc.tile_pool(name="owork", bufs=3) as opool,
        ):
            ph_t, th_t, xs_t = [], [], []
            for h in range(2):
                p = cpool.tile([HALF, L], F32, tag=f"ph{h}")
                nc.sync.dma_start(out=p[:], in_=phr[h])
                ph_t.append(p)
                t_ = cpool.tile([HALF, 1], F32, tag=f"th{h}")
                nc.sync.dma_start(out=t_[:], in_=thr[h])
                th_t.append(t_)
                xt = cpool.tile([HALF, B_SHARD], F32, tag=f"xs{h}")
                nc.sync.dma_start(out=xt[:], in_=xsr[h])
                xs_t.append(xt)

            def emit_chunk(h, ci):
                o = opool.tile([HALF, K * L], F16, tag="o")
                in0 = (
                    ph_t[h][:]
                    .rearrange("p (s n) -> p s n", s=1)
                    .to_broadcast((HALF, K, L))
                )
                in1 = (
                    xs_t[h][:, ci * K : (ci + 1) * K]
                    .rearrange("p (k n) -> p k n", n=1)
                    .to_broadcast((HALF, K, L))
                )
                nc.vector._custom_dve(
                    wincos,
                    out=o[:].rearrange("p (k n) -> p k n", n=L),
                    in0=in0,
                    in1=in1,
                    s0=th_t[h][:],
                    s1=FIT_A,
                )
                nc.sync.dma_start(
                    out=yr[h, :, ci * K : (ci + 1) * K, :], in_=o[:]
                )

            import contextlib

            loop_ctx = (
                tc.For_i(0, reps, 1, hint_engines=tuple(mybir.ALL_ENGINES))
                if reps > 1
                else contextlib.nullcontext()
            )
            with loop_ctx:
                for h in range(2):
                    for ci in range(n_chunks):
                        emit_chunk(h, ci)
    nc.compile()
    return nc


_NC_CACHE = {}


def _build_cfg():
    variant = os.environ.get("HB_VARIANT", "v3")
    K = int(os.environ.get("HB_K", "16"))
    return variant, K, 0.0, 0.0


def _get_nc():
    key = _build_cfg()
    if key not in _NC_CACHE:
        variant, K, _, _ = key
        _NC_CACHE[key] = build_nc(variant=variant, K=K)
    return _NC_CACHE[key]


def make_in_maps(x, phis, interval):
    """Host-side prep: scale streams by s, transpose x, shard over cores."""
    s = np.float32(SCALE_S)
    phs = np.ascontiguousarray(s * phis.astype(np.float32))
    thr_ = s * interval.astype(np.float32)
    thr_ = np.ascontiguousarray(thr_ * thr_)
    xs_full = s * x.astype(np.float32)
    in_maps = []
    for c in range(N_CORES):
        shard = xs_full[c * B_SHARD : (c + 1) * B_SHARD]
        in_maps.append(
            {
                "xs": np.ascontiguousarray(shard.T),
                "phs": phs,
                "thr": thr_,
            }
        )
    return in_maps


def kernel(x, phis, interval):
    x = np.ascontiguousarray(x, dtype=np.float32)
    phis = np.ascontiguousarray(phis, dtype=np.float32)
    interval = np.ascontiguousarray(interval, dtype=np.float32)
    assert x.shape == (B, M) and phis.shape == (M, L) and interval.shape == (M,)

    nc = _get_nc()
    in_maps = make_in_maps(x, phis, interval)
    res = run_bass_kernel_spmd(nc, in_maps, core_ids=list(range(N_CORES)))
    out = np.concatenate(
        [res.results[c]["out"] for c in range(N_CORES)], axis=0
    )
    return out.astype(np.float32)


# revision 9
# speedup vs baseline: 1276.7080x; 1276.7080x over previous
"""Trainium2 Bass kernel for histogram_binning (windowed-cosine binning).

Reference computation (per element):
    d = x[k,i] - phis[i,j]
    out[k, i*L+j] = 0.5*cos(d)+0.5  if  -interval[i] < d <= interval[i]  else 0

v3 strategy (8 cores, data-parallel over batch):
  - Single fused custom DVE op does EVERYTHING in one 1x pass (6 ALU
    stages), writing fp16; no ACT sin pass, no separate d-pass:
        d  = Src1 - Src0          # s*x - s*phi   (stream pre-scaled by s)
        v  = d * d                # = B * (d_true/2)^2  since s = sqrt(B)/2
        c  = A - v                # linear minimax fit of cos(d/2)
        m  = v <= T               # T = (s*iv)^2  (per-partition scalar)
        out = (m*c)^2             # = m * cos^2(d/2) = m*(0.5cos d + 0.5)
  - Src1 (x) is delivered per batch-row via a [P, K, L] access pattern
    with inner stride 0 (STT struct), so one instruction covers K rows:
    FD = K*L = 4096 -> per-instruction overhead amortized to ~1.5%.
  - Output fp16 halves HBM write traffic (47us/core) below the DVE pass
    (~69us/core), so the kernel is DVE-bound near the 1-pass floor.
    Host upcasts to f32 (well within the 2e-2 rel-err gate: measured
    ~5e-4 from the linear fit + fp16 quantization).
  - Mask boundary: v <= T on the scaled stream flips vs the reference's
    fl(x-phi) <= iv only within ~2ulp of the window edge (a handful of
    elements in 67M) and at d == -iv exactly (measure zero).
"""

import math
import os

import numpy as np

import concourse.bacc as bacc
import concourse.mybir as mybir
from concourse import dve_ops
from concourse.bass_utils import run_bass_kernel_spmd
from concourse.dve_spec import (
    C0,
    C1,
    C2,
    One,
    Spec,
    Src0,
    Src1,
    Zero,
    _has_src1,
    lower,
)
from concourse.dve_uop import DveOpSpec
from concourse.tile import TileContext

B, M, L = 1024, 256, 256
N_CORES = 8
B_SHARD = B // N_CORES  # 128
HALF = 128  # features per partition-half
F32 = mybir.dt.float32
F16 = {
    "fp16": mybir.dt.float16,
    "bf16": mybir.dt.bfloat16,
    "f32": mybir.dt.float32,
    "u8": mybir.dt.uint8,
}[os.environ.get("HB_ODT", "fp16")]
HALF_PI = float(np.pi / 2)

# minimax linear fit of cos(sqrt(t)) on t in [0, 0.25]:  cos(u) ~ A - B*u^2
FIT_A = 0.9996779888323271
FIT_B = 0.4896611490939912
SCALE_S = math.sqrt(FIT_B) / 2.0  # stream pre-scale: v = (s*d)^2 = B*(d/2)^2

_OPS_CACHE = {}


def _register_op(name, spec):
    """Register a custom DVE op under `name`, computing its uops sha."""
    if name in _OPS_CACHE:
        return _OPS_CACHE[name]
    for existing in dve_ops.OPS:
        if existing.name == name:
            _OPS_CACHE[name] = existing
            return existing
    if name not in dve_ops._SUB_OPCODE_FOR_NAME:
        row = max(dve_ops._SUB_OPCODE_FOR_NAME.values()) + 1
        assert row < 0x20, "no free custom-DVE opcode rows"
        dve_ops._SUB_OPCODE_FOR_NAME[name] = row
    shas = {}
    for ver in ("v3", "v4"):
        uops = lower(spec, ver=ver)
        shas[ver] = DveOpSpec(
            name=name,
            opcode=dve_ops.get_dve_sub_opcode(name),
            uops=uops,
            rd1_en=_has_src1(spec),
        ).sha(ver)
    op = dve_ops.DveOp(name, spec, subdim=False, uops_sha=shas)
    dve_ops.OPS.append(op)
    dve_ops.CUSTOM_DVE_SPECS[name] = spec
    _OPS_CACHE[name] = op
    return op


def _get_wincos_op():
    """out = (m*(A - v))^2 with v = (Src1-Src0)^2, m = v <= C0.
    Src0 = s*phi stream, Src1 = s*x (paged, inner stride 0),
    C0 (s0) = (s*iv)^2 per-partition, C1 (s1) = A literal."""
    d = Src1 - Src0
    v = d * d
    c = C1 - v
    m = v <= C0
    mc = m * c
    body = mc * mc

    def _ref(in0, in1, s0, s1, imm2):
        f = np.float32
        dd = (in1 - in0).astype(np.float32)
        vv = (dd * dd).astype(np.float32)
        mm = (vv <= s0).astype(np.float32)
        cc = (f(s1) - vv).astype(np.float32)
        r = (mm * cc).astype(np.float32)
        return (r * r).astype(np.float32)

    return _register_op("WINCOS_FUSED_ANT", Spec(body=body, reference=_ref))


def _get_wincos_perk_op():
    """Per-k TTSS variant: x arrives as per-partition scalar C1, no Src1.
    out = (m*(A - v))^2, v = (C1-Src0)^2, m = v <= C0, C2(imm2) = A."""
    d = C1 - Src0
    v = d * d
    c = C2 - v
    m = v <= C0
    mc = m * c
    body = mc * mc

    def _ref(in0, in1, s0, s1, imm2):
        f = np.float32
        dd = (np.float32(s1) - in0).astype(np.float32)
        vv = (dd * dd).astype(np.float32)
        mm = (vv <= s0).astype(np.float32)
        cc = (f(imm2) - vv).astype(np.float32)
        r = (mm * cc).astype(np.float32)
        return (r * r).astype(np.float32)

    return _register_op("WINCOS_PERK_ANT", Spec(body=body, reference=_ref))


def build_nc(variant="v3", K=16, num_devices=N_CORES, reps=1, mode="full", **_unused):
    """Build the per-core Bass program (v3: single fused custom op).

    mode: "full" | "nodma" (compute only, one token DMA per rep) |
          "dmaonly" (no compute, same output-DMA pattern)."""
    assert B_SHARD % K == 0
    n_chunks = B_SHARD // K

    nc = bacc.Bacc(
        "TRN2",
        target_bir_lowering=False,
        debug=False,
        enable_asserts=True,
        num_devices=num_devices,
    )
    # host-prescaled inputs: xs = s*x^T [M, B_SHARD], phs = s*phis [M, L],
    # thr = (s*interval)^2 [M]
    xs_d = nc.dram_tensor("xs", [M, B_SHARD], F32, kind="ExternalInput")
    ph_d = nc.dram_tensor("phs", [M, L], F32, kind="ExternalInput")
    th_d = nc.dram_tensor("thr", [M], F32, kind="ExternalInput")
    y_d = nc.dram_tensor("out", [B_SHARD, M * L], F16, kind="ExternalOutput")
    # out[k, (h*128+i)*256 + j] viewed as [h, i(part), k, j]
    yr = y_d.ap().rearrange("b (h i j) -> h i b j", h=2, i=HALF, j=L)
    thr = th_d.ap().rearrange("(h i one) -> h i one", h=2, one=1)
    xsr = xs_d.ap().rearrange("(h i) b -> h i b", h=2)
    phr = ph_d.ap().rearrange("(h i) j -> h i j", h=2)

    wincos = _get_wincos_perk_op() if variant == "v3k" else _get_wincos_op()

    with TileContext(nc) as tc:
        with (
            tc.tile_pool(name="const", bufs=1) as cpool,
            tc.tile_pool(name="owork", bufs=3) as opool,
        ):
            ph_t, th_t, xs_t = [], [], []
            for h in range(2):
                p = cpool.tile([HALF, L], F32, tag=f"ph{h}")
                nc.sync.dma_start(out=p[:], in_=phr[h])
                ph_t.append(p)
                t_ = cpool.tile([HALF, 1], F32, tag=f"th{h}")
                nc.sync.dma_start(out=t_[:], in_=thr[h])
                th_t.append(t_)
                xt = cpool.tile([HALF, B_SHARD], F32, tag=f"xs{h}")
                nc.sync.dma_start(out=xt[:], in_=xsr[h])
                xs_t.append(xt)

            dma_src = None
            if mode == "dmaonly":
                dma_src = cpool.tile([HALF, K * L], F16, tag="dsrc")
                nc.gpsimd.memset(dma_src[:], 0.25)

            def emit_chunk(h, ci):
                if mode == "dmaonly":
                    nc.sync.dma_start(
                        out=yr[h, :, ci * K : (ci + 1) * K, :], in_=dma_src[:]
                    )
                    return
                o = opool.tile([HALF, K * L], F16, tag="o")
                if variant == "v3k":
                    for k in range(K):
                        kg = ci * K + k
                        nc.vector._custom_dve(
                            wincos,
                            out=o[:, k * L : (k + 1) * L],
                            in0=ph_t[h][:],
                            s0=th_t[h][:],
                            s1=xs_t[h][:, kg : kg + 1],
                            imm2=FIT_A,
                        )
                else:
                    in0 = (
                        ph_t[h][:]
                        .rearrange("p (s n) -> p s n", s=1)
                        .to_broadcast((HALF, K, L))
                    )
                    in1 = (
                        xs_t[h][:, ci * K : (ci + 1) * K]
                        .rearrange("p (k n) -> p k n", n=1)
                        .to_broadcast((HALF, K, L))
                    )
                    nc.vector._custom_dve(
                        wincos,
                        out=o[:].rearrange("p (k n) -> p k n", n=L),
                        in0=in0,
                        in1=in1,
                        s0=th_t[h][:],
                        s1=FIT_A,
                    )
                if mode == "full":
                    nc.sync.dma_start(
                        out=yr[h, :, ci * K : (ci + 1) * K, :], in_=o[:]
                    )
                elif ci == n_chunks - 1:
                    # token DMA so every rep's last tile is consumed
                    nc.sync.dma_start(
                        out=yr[h, :, 0:1, :], in_=o[:, 0:L]
                    )

            # reps are UNROLLED (no tc.For_i): hardware loops push the
            # output DMAs off the static-HWDGE path and per-iteration
            # SWDGE descriptor generation dominates (~1.25ms/iter).
            for _ in range(reps):
                for h in range(2):
                    for ci in range(n_chunks):
                        emit_chunk(h, ci)
    nc.compile()
    return nc


_NC_CACHE = {}


def _build_cfg():
    variant = os.environ.get("HB_VARIANT", "v3")
    K = int(os.environ.get("HB_K", "16"))
    return variant, K, 0.0, 0.0


def _get_nc():
    key = _build_cfg()
    if key not in _NC_CACHE:
        variant, K, _, _ = key
        _NC_CACHE[key] = build_nc(variant=variant, K=K)
    return _NC_CACHE[key]


def make_in_maps(x, phis, interval):
    """Host-side prep: scale streams by s, transpose x, shard over cores."""
    s = np.float32(SCALE_S)
    phs = np.ascontiguousarray(s * phis.astype(np.float32))
    thr_ = s * interval.astype(np.float32)
    thr_ = np.ascontiguousarray(thr_ * thr_)
    xs_full = s * x.astype(np.float32)
    in_maps = []
    for c in range(N_CORES):
        shard = xs_full[c * B_SHARD : (c + 1) * B_SHARD]
        in_maps.append(
            {
                "xs": np.ascontiguousarray(shard.T),
                "phs": phs,
                "thr": thr_,
            }
        )
    return in_maps


def kernel(x, phis, interval):
    x = np.ascontiguousarray(x, dtype=np.float32)
    phis = np.ascontiguousarray(phis, dtype=np.float32)
    interval = np.ascontiguousarray(interval, dtype=np.float32)
    assert x.shape == (B, M) and phis.shape == (M, L) and interval.shape == (M,)

    nc = _get_nc()
    in_maps = make_in_maps(x, phis, interval)
    res = run_bass_kernel_spmd(nc, in_maps, core_ids=list(range(N_CORES)))
    out = np.concatenate(
        [res.results[c]["out"] for c in range(N_CORES)], axis=0
    )
    return out.astype(np.float32)
